# revision 30
# baseline (speedup 1.0000x reference)
"""AGNN (3-layer cosine-attention message passing) on 8 trn2 NeuronCores.

Self-contained: host-side graph prep (numpy) + Bass/Tile device program +
run via a cached PJRT executable. kernel(**inputs) takes the full
unsharded inputs and returns the full [G, C] output.

Sharding: nodes (and their incoming edges) are partitioned across the 8
cores by dst. Host ships only the per-core node features (fp8-e3m4) and
compact edge indices (u16 src row + i8 dst slot); everything else is
built on device:
  - each core widens fp8->f16, normalizes its rows and packs
    [nh*sqrt(beta) | h | 1] table rows; an AllGather replicates the table,
  - per edge tile, src rows are fetched by indirect DMA from the
    replicated table; dst nh rows are reconstructed on the PE by
    transposing the edge mask and multiplying with the local nh block,
  - edge softmax + scatter-by-dst run as masked matmuls into PSUM,
  - an AllGather rebuilds the replicated table between layers.
The final graph readout (mean-pool partials) is summed on host and put
through the tiny [64,128]@[128,100] classifier in numpy.
"""

import sys

sys.path.insert(0, "/opt/trn_rl_repo")

import numpy as np
import ml_dtypes

import concourse.bass as bass
import concourse.bacc as bacc
import concourse.mybir as mybir
import concourse.tile as tile

EPS = 1e-12


# ---------------------------------------------------------------- config

class Cfg:
    def __init__(self, N, E, G, NC, blocks_per_sb=3):
        self.N = N                    # real nodes
        self.E = E                    # edges
        self.G = G                    # graphs
        self.NC = NC                  # cores
        self.D = 128
        self.NPC = N // NC            # real nodes per core
        self.BLK = 128
        self.NBLK = -(-self.NPC // self.BLK)       # blocks per core
        self.NPAD = self.NBLK * self.BLK           # padded nodes per core
        self.NPADTOT = self.NPAD * NC
        self.ROW = 258                # [nh 128 | h 128 | 1 | pad]
        self.SBS = blocks_per_sb      # dst blocks per super-block
        self.NSB = -(-self.NBLK // self.SBS)
        self.L = 3
        self.W = 16                   # per-core graph-id window (sorted ids
                                      # mean each core's nodes span few graphs)


# ---------------------------------------------------------------- host prep

def _prep(cfg, h, src, dst, graph_ids, betas):
    """Build per-core input maps + the shared tile schedule."""
    N, NC, NPC, NPAD, BLK, NBLK = cfg.N, cfg.NC, cfg.NPC, cfg.NPAD, cfg.BLK, cfg.NBLK
    h = np.asarray(h, np.float32)
    src = np.asarray(src, np.int64)
    dst = np.asarray(dst, np.int64)
    graph_ids = np.asarray(graph_ids, np.int64)
    betas = np.asarray(betas, np.float32)

    # edges sorted by global dst -> grouped by (core, local block)
    order = np.argsort(dst, kind="stable")
    e_src = src[order]
    e_dst = dst[order]
    src_pad = (e_src // NPC) * NPAD + (e_src % NPC)
    dcore = e_dst // NPC
    dlocal = e_dst % NPC
    dblk = dlocal // BLK

    # per (core, block) edge counts -> shared tile schedule
    cnt = np.zeros((NC, NBLK), np.int64)
    np.add.at(cnt, (dcore, dblk), 1)
    T_b = np.maximum(1, -(-cnt.max(0) // 128))     # tiles per block (shared)
    Ttot = int(T_b.sum())
    tcol0 = np.zeros(NBLK, np.int64)               # first tile col per block
    tcol0[1:] = np.cumsum(T_b)[:-1]

    sqbeta = np.zeros((128, 4), np.float32)
    for l in range(min(3, len(betas))):
        sqbeta[:, l] = np.sqrt(betas[l])

    # within-(core,block) rank of each edge (edges are sorted by dst so
    # (core, block) groups are contiguous and in order)
    blk_first = np.zeros((NC, NBLK), np.int64)     # first edge idx per group
    flat_cnt = cnt.reshape(-1)
    blk_first.reshape(-1)[1:] = np.cumsum(flat_cnt)[:-1]
    rank = np.arange(len(e_dst)) - blk_first[dcore, dblk]
    ecol = tcol0[dblk] + rank // 128               # tile column of each edge
    erow = rank % 128                              # tile row of each edge

    # one contiguous u8 blob per core: [h8 | srcidx u16 | drel i8 | gid | sqbeta]
    nbytes = NPC * 128 + 384 * Ttot + 128 * NBLK + 2048
    blob = np.zeros((NC, nbytes), np.uint8)
    w0s = []
    for c in range(NC):
        dummy = c * NPAD + NPC                     # an all-zero table row
        srcidx = np.full((128, Ttot), dummy, np.uint16)
        drel = np.full((128, Ttot), -1, np.int8)   # pad edges: mask misses

        m = dcore == c
        srcidx[erow[m], ecol[m]] = src_pad[m]
        drel[erow[m], ecol[m]] = dlocal[m] - dblk[m] * BLK

        h8 = h[c * NPC:(c + 1) * NPC].astype(ml_dtypes.float8_e3m4)

        gl = graph_ids[c * NPC:(c + 1) * NPC]
        w0 = int(gl[0])
        if int(gl[-1]) - w0 >= cfg.W:
            raise ValueError(
                f"core {c} graph span {int(gl[-1]) - w0 + 1} exceeds window "
                f"{cfg.W}; widen Cfg.W")
        w0s.append(w0)
        gid = np.full(NPAD, -1, np.int8)
        gid[:NPC] = gl - w0
        gid = np.ascontiguousarray(gid.reshape(NBLK, 128).T)  # [128, NBLK]

        parts = [h8.view(np.uint8).reshape(-1), srcidx.view(np.uint8).reshape(-1),
                 drel.view(np.uint8).reshape(-1), gid.view(np.uint8).reshape(-1),
                 sqbeta.view(np.uint8).reshape(-1)]
        blob[c] = np.concatenate(parts)

    counts = np.bincount(graph_ids, minlength=cfg.G).astype(np.float32)
    sched = dict(T_b=[int(x) for x in T_b], tcol0=[int(x) for x in tcol0],
                 Ttot=Ttot, nbytes=nbytes, w0s=w0s)
    return blob.reshape(NC * nbytes), counts, sched


# ---------------------------------------------------------------- device program

def build_program(cfg, sched, trace_sim=False):
    f16, f32, i32 = mybir.dt.float16, mybir.dt.float32, mybir.dt.int32
    u16, i8, f8 = mybir.dt.uint16, mybir.dt.int8, mybir.dt.float8e3
    T_b, tcol0 = sched["T_b"], sched["tcol0"]
    Ttot = sched["Ttot"]
    NBLK, SBS, NSB, ROW, G = cfg.NBLK, cfg.SBS, cfg.NSB, cfg.ROW, cfg.G
    Tmax = max(sum(T_b[sb * SBS:(sb + 1) * SBS]) for sb in range(NSB))

    nc = bacc.Bacc("TRN2", target_bir_lowering=False, debug=False,
                   num_devices=cfg.NC)

    nbytes = sched["nbytes"]
    W, NPC = cfg.W, cfg.NPC
    blob_d = nc.dram_tensor("blob", [nbytes], mybir.dt.uint8,
                            kind="ExternalInput").ap()
    o0 = NPC * 128
    o1 = o0 + 256 * Ttot
    o2 = o1 + 128 * Ttot
    o3 = o2 + 128 * NBLK
    h8_d = blob_d[0:o0].bitcast(f8).rearrange("(n d) -> n d", d=128)
    srcidx_d = blob_d[o0:o1].bitcast(u16).rearrange("(p t) -> p t", t=Ttot)
    drel_d = blob_d[o1:o2].bitcast(i8).rearrange("(p t) -> p t", t=Ttot)
    gid_d = blob_d[o2:o3].bitcast(i8).rearrange("(p b) -> p b", b=NBLK)
    sqbeta_d = blob_d[o3:o3 + 2048].bitcast(f32).rearrange("(p x) -> p x", x=4)
    pooled_d = nc.dram_tensor("pooled", [W, 128], f16, kind="ExternalOutput").ap()

    own = [nc.dram_tensor(f"own{l}", [cfg.NPAD, ROW], f16).ap()
           for l in range(cfg.L)]
    tab_space = "Shared" if cfg.NC > 4 else "Local"
    tabs = [nc.dram_tensor(f"tab{l}", [cfg.NPADTOT, ROW], f16,
                           addr_space=tab_space).ap()
            for l in range(cfg.L)]

    groups = [list(range(cfg.NC))]

    from contextlib import ExitStack

    with tile.TileContext(nc, trace_sim=trace_sim) as tc, ExitStack() as ctx:
        const = ctx.enter_context(tc.tile_pool(name="const", bufs=1))
        iota_i = const.tile([128, 128], i32)
        nc.gpsimd.iota(iota_i[:], pattern=[[1, 128]], base=0, channel_multiplier=0)
        iota_f = const.tile([128, 128], f16)
        nc.vector.tensor_copy(iota_f[:], iota_i[:])
        iotac_i = const.tile([128, 1], i32)
        nc.gpsimd.iota(iotac_i[:], pattern=[[0, 1]], base=0, channel_multiplier=1)
        iotac_f = const.tile([128, 1], f16)
        nc.vector.tensor_copy(iotac_f[:], iotac_i[:])
        # identity matrix for PE transposes: I[p, j] = (j == p)
        ident = const.tile([128, 128], f16)
        ic_b = iotac_f[:].rearrange("p (a b) -> p a b", a=1) \
            .to_broadcast([128, 1, 128])
        io_r = iota_f[:].rearrange("p (a b) -> p a b", a=1)
        nc.vector.tensor_tensor(
            out=ident[:].rearrange("p (a b) -> p a b", a=1),
            in0=io_r, in1=ic_b, op=mybir.AluOpType.is_equal)
        sqbeta = const.tile([128, 4], f32)
        nc.sync.dma_start(sqbeta[:], sqbeta_d)

        # selg[p, b*W + j] = (local_graph_id[b*128 + p] == j)
        gid8 = const.tile([128, NBLK], i8)
        nc.sync.dma_start(gid8[:], gid_d)
        selg_s = const.tile([128, NBLK * W], f16)
        sg3 = selg_s[:].rearrange("p (b j) -> p b j", j=W)
        io_g = iota_f[:, 0:W].rearrange("p (o j) -> p o j", o=1) \
            .to_broadcast([128, NBLK, W])
        gid_b = gid8[:].rearrange("p (b o) -> p b o", o=1) \
            .to_broadcast([128, NBLK, W])
        nc.vector.tensor_tensor(out=sg3, in0=io_g, in1=gid_b,
                                op=mybir.AluOpType.is_equal)

        idxp = ctx.enter_context(tc.tile_pool(name="idxp", bufs=3))
        gp = ctx.enter_context(tc.tile_pool(name="gp", bufs=2))
        cp = ctx.enter_context(tc.tile_pool(name="cp", bufs=2))
        ep = ctx.enter_context(tc.tile_pool(name="ep", bufs=2))
        pp = ctx.enter_context(tc.tile_pool(name="pp", bufs=2, space="PSUM"))
        pt = ctx.enter_context(tc.tile_pool(name="pt", bufs=2, space="PSUM"))
        pq = ctx.enter_context(tc.tile_pool(name="pq", bufs=2, space="PSUM"))
        ppool = ctx.enter_context(tc.tile_pool(name="ppool", bufs=1, space="PSUM"))

        pool_ps = ppool.tile([W, 128], f32, tag="pool")

        def build_rows(h3, out_dram, sb, nb, lidx):
            """Pack [nh*sqrt(beta_lidx) | h | 1] rows of super-block sb and
            DMA them to out_dram. h3: [128, nb, 128] f32 or f16 tile view."""
            sq = ep.tile([128, SBS * 128], f32, tag="sq")
            q3 = sq[:, 0:nb * 128].rearrange("p (b d) -> p b d", d=128)
            nc.vector.tensor_tensor(out=q3, in0=h3, in1=h3,
                                    op=mybir.AluOpType.mult)
            ss = ep.tile([128, SBS], f32, tag="ss")
            nc.vector.tensor_reduce(
                out=ss[:, 0:nb], in_=q3, axis=mybir.AxisListType.X,
                op=mybir.AluOpType.add)
            nrm = ep.tile([128, SBS], f32, tag="nrm")
            nc.scalar.sqrt(nrm[:, 0:nb], ss[:, 0:nb])
            nc.vector.tensor_scalar_add(nrm[:, 0:nb], nrm[:, 0:nb], EPS)
            rn = ep.tile([128, SBS], f32, tag="rn")
            nc.vector.reciprocal(rn[:, 0:nb], nrm[:, 0:nb])

            stg = ep.tile([128, SBS * ROW], f16, tag="stg")
            st3 = stg[:, 0:nb * ROW].rearrange("p (b d) -> p b d", d=ROW)
            rn_b = rn[:, 0:nb].rearrange("p (b o) -> p b o", o=1) \
                .to_broadcast([128, nb, 128])
            nc.vector.scalar_tensor_tensor(
                out=st3[:, :, 0:128], in0=h3,
                scalar=sqbeta[:, lidx:lidx + 1], in1=rn_b,
                op0=mybir.AluOpType.mult, op1=mybir.AluOpType.mult)
            nc.scalar.copy(out=st3[:, :, 128:256], in_=h3)
            nc.vector.memset(st3[:, :, 256:258], 1.0)
            out_ap = out_dram[sb * SBS * 128: sb * SBS * 128 + nb * 128, :] \
                .rearrange("(b p) d -> p b d", p=128)
            nc.sync.dma_start(out_ap, st3)

        # ---- layer-0 table build: widen fp8, normalize + pack, AllGather
        # (h8 ships only the NPC real rows; the tail of the last super-block
        # is zero-filled so pad rows become all-zero table rows)
        for sb in range(NSB):
            blocks = list(range(sb * SBS, min((sb + 1) * SBS, NBLK)))
            nb = len(blocks)
            r0 = sb * SBS * 128
            nreal = min(nb * 128, NPC - r0)
            nfull = nreal // 128
            rem = nreal % 128
            hin8 = ep.tile([128, SBS * 128], f8, tag="hin8")
            h83 = hin8[:, 0:nb * 128].rearrange("p (b d) -> p b d", d=128)
            hin = ep.tile([128, SBS * 128], f16, tag="hin")
            hi3 = hin[:, 0:nb * 128].rearrange("p (b d) -> p b d", d=128)
            if nreal < nb * 128:
                nc.vector.memset(hi3, 0.0)
            if nfull:
                nc.sync.dma_start(
                    h83[:, 0:nfull, :],
                    h8_d[r0: r0 + nfull * 128, :]
                    .rearrange("(b p) d -> p b d", p=128))
                # fp8e3 -> f16 must run on the scalar engine (DVE e3m4 path
                # is broken on HW)
                nc.scalar.copy(out=hi3[:, 0:nfull, :], in_=h83[:, 0:nfull, :])
            if rem:
                nc.sync.dma_start(
                    h83[0:rem, nfull, :],
                    h8_d[r0 + nfull * 128: r0 + nreal, :])
                nc.scalar.copy(out=hi3[0:rem, nfull, :],
                               in_=h83[0:rem, nfull, :])
            build_rows(hi3, own[0], sb, nb, 0)
        nc.gpsimd.collective_compute(
            "AllGather", mybir.AluOpType.bypass, replica_groups=groups,
            ins=[own[0][:, :]], outs=[tabs[0][:, :]])

        for l in range(cfg.L):
            tab = tabs[l]
            for sb in range(NSB):
                blocks = list(range(sb * SBS, min((sb + 1) * SBS, NBLK)))
                nb = len(blocks)
                c0 = tcol0[blocks[0]]
                Tsb = sum(T_b[b] for b in blocks)

                tile_bi = []
                for bi, b in enumerate(blocks):
                    tile_bi += [bi] * T_b[b]

                # ---- indices (u16 -> i32 on device) + dst-rel positions
                idxu = idxp.tile([128, Tmax], u16, tag="idxu")
                nc.sync.dma_start(idxu[:, 0:Tsb], srcidx_d[:, c0:c0 + Tsb])
                idx_s = idxp.tile([128, Tmax], i32, tag="idxs")
                nc.vector.tensor_copy(idx_s[:, 0:Tsb], idxu[:, 0:Tsb])
                drel = idxp.tile([128, Tmax], i8, tag="drel")
                nc.sync.dma_start(drel[:, 0:Tsb], drel_d[:, c0:c0 + Tsb])

                # ---- local dst nh rows for this super-block
                nhblk = idxp.tile([128, SBS * 128], f16, tag="nhblk")
                nb3 = nhblk[:, 0:nb * 128].rearrange("p (b d) -> p b d", d=128)
                nc.sync.dma_start(
                    nb3,
                    own[l][sb * SBS * 128: sb * SBS * 128 + nb * 128, 0:128]
                    .rearrange("(b p) d -> p b d", p=128))

                # ---- src gather: one [128,1]-offset call per 128-edge tile
                # (HW contract: partition p reads a contiguous line from
                # row idx[p]; multi-column offset APs are NOT honored)
                gsrc = gp.tile([128, Tmax * ROW], f16, tag="gsrc")
                for t in range(Tsb):
                    nc.gpsimd.indirect_dma_start(
                        out=gsrc[:, t * ROW:(t + 1) * ROW], out_offset=None,
                        in_=tab, in_offset=bass.IndirectOffsetOnAxis(
                            ap=idx_s[:, t:t + 1], axis=0))
                g3 = gsrc[:, 0:Tsb * ROW].rearrange("p (t d) -> p t d", d=ROW)

                # ---- edge mask:  sel[e, t, j] = (iota[j] == drel[e, t])
                sel = gp.tile([128, Tmax * 128], f16, tag="sel")
                s3 = sel[:, 0:Tsb * 128].rearrange("p (t j) -> p t j", j=128)
                io_b = iota_f[:].rearrange("p (o j) -> p o j", o=1) \
                    .to_broadcast([128, Tsb, 128])
                dr_b = drel[:, 0:Tsb].rearrange("p (t o) -> p t o", o=1) \
                    .to_broadcast([128, Tsb, 128])
                nc.vector.tensor_tensor(
                    out=s3, in0=io_b, in1=dr_b, op=mybir.AluOpType.is_equal)

                # ---- scores: transpose the mask on the PE, pull each
                # edge's dst nh row, then s = <nh_src, nh_dst>, a = exp(s)
                s_t = cp.tile([128, Tmax], f32, tag="s")
                for g0 in range(0, Tsb, 4):
                    gn = min(4, Tsb - g0)
                    mT = pt.tile([128, 512], f32, tag="mT")
                    for k in range(gn):
                        nc.tensor.matmul(
                            out=mT[:, k * 128:(k + 1) * 128],
                            lhsT=s3[:, g0 + k, :], rhs=ident[:],
                            start=True, stop=True)
                    sT = cp.tile([128, 512], f16, tag="sT")
                    nc.scalar.copy(out=sT[:, 0:gn * 128], in_=mT[:, 0:gn * 128])
                    px = pq.tile([128, 512], f32, tag="px")
                    for k in range(gn):
                        nc.tensor.matmul(
                            out=px[:, k * 128:(k + 1) * 128],
                            lhsT=sT[:, k * 128:(k + 1) * 128],
                            rhs=nb3[:, tile_bi[g0 + k], :],
                            start=True, stop=True)
                    prod = cp.tile([128, 512], f16, tag="prod")
                    pr3 = prod[:, 0:gn * 128].rearrange("p (t d) -> p t d", d=128)
                    nc.vector.tensor_tensor(
                        out=pr3,
                        in0=px[:, 0:gn * 128].rearrange("p (t d) -> p t d", d=128),
                        in1=g3[:, g0:g0 + gn, 0:128], op=mybir.AluOpType.mult)
                    nc.vector.tensor_reduce(
                        out=s_t[:, g0:g0 + gn], in_=pr3,
                        axis=mybir.AxisListType.X, op=mybir.AluOpType.add)
                a_t = cp.tile([128, Tmax], f16, tag="a")
                nc.scalar.activation(
                    out=a_t[:, 0:Tsb], in_=s_t[:, 0:Tsb],
                    func=mybir.ActivationFunctionType.Exp)

                # ---- attention weights:  sel *= a  (in place)
                a_b = a_t[:, 0:Tsb].rearrange("p (t o) -> p t o", o=1) \
                    .to_broadcast([128, Tsb, 128])
                nc.vector.tensor_tensor(
                    out=s3, in0=s3, in1=a_b, op=mybir.AluOpType.mult)

                # ---- scatter:  psum[:, bb*129:(bb+1)*129] += asel_t^T @ [h|1]
                pn = pp.tile([128, 512], f32, tag="pn")
                tt = 0
                for bi, b in enumerate(blocks):
                    for t in range(T_b[b]):
                        nc.tensor.matmul(
                            out=pn[:, bi * 129:bi * 129 + 129],
                            lhsT=s3[:, tt, :],
                            rhs=g3[:, tt, 128:257],
                            start=(t == 0), stop=(t == T_b[b] - 1))
                        tt += 1

                # ---- epilogue: h' = num / max(den, tiny)
                p3 = pn[:, 0:nb * 129].rearrange("p (b d) -> p b d", d=129)
                den = ep.tile([128, SBS], f32, tag="den")
                nc.vector.tensor_scalar_max(den[:, 0:nb], p3[:, :, 128:129], 1e-30)
                rec = ep.tile([128, SBS], f32, tag="rec")
                nc.vector.reciprocal(rec[:, 0:nb], den[:, 0:nb])
                hsb = ep.tile([128, SBS * 128], f32, tag="hsb")
                h3 = hsb[:, 0:nb * 128].rearrange("p (b d) -> p b d", d=128)
                rec_b = rec[:, 0:nb].rearrange("p (b o) -> p b o", o=1) \
                    .to_broadcast([128, nb, 128])
                nc.vector.tensor_tensor(
                    out=h3, in0=p3[:, :, 0:128], in1=rec_b,
                    op=mybir.AluOpType.mult)

                if l < cfg.L - 1:
                    build_rows(h3, own[l + 1], sb, nb, l + 1)
                else:
                    hf = ep.tile([128, SBS * 128], f16, tag="hf")
                    hf3 = hf[:, 0:nb * 128].rearrange("p (b d) -> p b d", d=128)
                    nc.scalar.copy(out=hf3, in_=h3)
                    for bi, b in enumerate(blocks):
                        nc.tensor.matmul(
                            out=pool_ps[:, :],
                            lhsT=selg_s[:, b * W:b * W + W],
                            rhs=hf3[:, bi, :],
                            start=(b == 0), stop=(b == NBLK - 1))

            if l < cfg.L - 1:
                nc.gpsimd.collective_compute(
                    "AllGather", mybir.AluOpType.bypass,
                    replica_groups=groups,
                    ins=[own[l + 1][:, :]], outs=[tabs[l + 1][:, :]])

        pooled_s = const.tile([W, 128], f16)
        nc.scalar.copy(out=pooled_s[:, :], in_=pool_ps[:, :])
        nc.sync.dma_start(pooled_d, pooled_s[:, :])

    return nc


# ---------------------------------------------------------------- runner

LAST_EXEC_NS = None
_CACHE = {}


def _build_runner(nc, n_cores):
    """A cached PJRT executable for nc: jit once, reuse across runs.
    Each call still uploads the full per-core inputs and downloads the
    outputs (matching run_bass_kernel_spmd's per-call semantics)."""
    import jax
    from jax.sharding import Mesh, PartitionSpec
    from jax.experimental.shard_map import shard_map
    from concourse.bass2jax import (
        _bass_exec_p, partition_id_tensor, install_neuronx_cc_hook)

    install_neuronx_cc_hook()
    assert nc.dbg_addr is None or not nc.dbg_callbacks

    partition_name = nc.partition_id_tensor.name if nc.partition_id_tensor else None
    in_names, out_names, out_avals = [], [], []
    for alloc in nc.m.functions[0].allocations:
        if not isinstance(alloc, mybir.MemoryLocationSet):
            continue
        name = alloc.memorylocations[0].name
        if alloc.kind == "ExternalInput":
            if name != partition_name:
                in_names.append(name)
        elif alloc.kind == "ExternalOutput":
            out_names.append(name)
            out_avals.append(jax.core.ShapedArray(
                tuple(alloc.tensor_shape), mybir.dt.np(alloc.dtype)))
    n_params = len(in_names)
    n_outs = len(out_avals)
    all_in_names = in_names + out_names + (
        [partition_name] if partition_name else [])
    donate = tuple(range(n_params, n_params + n_outs))

    def _body(*args):
        operands = list(args)
        if partition_name is not None:
            operands.append(partition_id_tensor())
        outs = _bass_exec_p.bind(
            *operands, out_avals=tuple(out_avals), in_names=tuple(all_in_names),
            out_names=tuple(out_names), lowering_input_output_aliases=(),
            sim_require_finite=True, sim_require_nnan=True, nc=nc)
        return tuple(outs)

    devices = jax.devices()[:n_cores]
    mesh = Mesh(np.asarray(devices), ("core",))
    sharded = jax.jit(
        shard_map(_body, mesh=mesh,
                  in_specs=(PartitionSpec("core"),) * (n_params + n_outs),
                  out_specs=(PartitionSpec("core"),) * n_outs,
                  check_rep=False),
        donate_argnums=donate, keep_unused=True)

    def run(full_map):
        """full_map: input name -> pre-concatenated (n_cores*dim0, ...) array."""
        concat_in = [np.asarray(full_map[name]) for name in in_names]
        concat_zeros = [
            np.zeros((n_cores * a.shape[0], *a.shape[1:]), a.dtype)
            for a in out_avals]
        out_arrs = sharded(*concat_in, *concat_zeros)
        return [
            {name: np.asarray(out_arrs[i]).reshape(
                n_cores, *out_avals[i].shape)[c]
             for i, name in enumerate(out_names)}
            for c in range(n_cores)]

    return run


def _get_runner(cfg, sched):
    key = tuple(sched["T_b"])
    if key not in _CACHE:
        nc = build_program(cfg, sched)
        nc.compile()
        _CACHE[key] = (nc, _build_runner(nc, cfg.NC))
    return _CACHE[key][1]


# ---------------------------------------------------------------- entry

def kernel(h, src, dst, graph_ids, betas, W_cls, b_cls, time_execs=0):
    global LAST_EXEC_NS
    import time as _time

    cfg = Cfg(N=40000, E=640000, G=64, NC=8)
    blob, counts, sched = _prep(cfg, h, src, dst, graph_ids, betas)
    run = _get_runner(cfg, sched)
    full_map = {"blob": blob}

    def _run():
        last = None
        for attempt in range(3):
            try:
                return run(full_map)
            except Exception as e:  # transient axon worker hangs
                last = e
                _time.sleep(5)
        raise last

    res = _run()
    if time_execs:
        # no NTFF profiling hook is available in this container, so report
        # median wall-clock of repeated NEFF executions (includes the axon
        # dispatch + input-upload overhead; on-device time is lower)
        ts = []
        for _ in range(time_execs):
            t0 = _time.time()
            res = run(full_map)
            ts.append(_time.time() - t0)
        LAST_EXEC_NS = int(np.median(ts) * 1e9)
    pooled = np.zeros((cfg.G, 128), np.float64)
    for c, r in enumerate(res):
        lo = sched["w0s"][c]
        hi = min(lo + cfg.W, cfg.G)
        pooled[lo:hi] += r["pooled"][:hi - lo].astype(np.float64)
    hg = (pooled / np.maximum(counts, 1.0)[:, None]).astype(np.float32)
    return hg @ np.asarray(W_cls, np.float32) + np.asarray(b_cls, np.float32)


# revision 31
# speedup vs baseline: 1.0598x; 1.0598x over previous
"""AGNN (3-layer cosine-attention message passing) on 8 trn2 NeuronCores.

Self-contained: host-side graph prep (numpy) + Bass/Tile device program +
run via a cached PJRT executable. kernel(**inputs) takes the full
unsharded inputs and returns the full [G, C] output.

Sharding: nodes (and their incoming edges) are partitioned across the 8
cores by dst. Host ships only the per-core node features (fp8-e3m4) and
compact edge indices (u16 src row + i8 dst slot); everything else is
built on device:
  - each core widens fp8->f16, normalizes its rows and packs
    [nh*sqrt(beta) | h | 1] table rows; an AllGather replicates the table,
  - per edge tile, src rows are fetched by indirect DMA from the
    replicated table; dst nh rows are reconstructed on the PE by
    transposing the edge mask and multiplying with the local nh block,
  - edge softmax + scatter-by-dst run as masked matmuls into PSUM,
  - an AllGather rebuilds the replicated table between layers.
The final graph readout (mean-pool partials) is summed on host and put
through the tiny [64,128]@[128,100] classifier in numpy.
"""

import sys

sys.path.insert(0, "/opt/trn_rl_repo")

import numpy as np
import ml_dtypes

import concourse.bass as bass
import concourse.bacc as bacc
import concourse.mybir as mybir
import concourse.tile as tile

EPS = 1e-12


# ---------------------------------------------------------------- config

class Cfg:
    def __init__(self, N, E, G, NC, blocks_per_sb=3):
        self.N = N                    # real nodes
        self.E = E                    # edges
        self.G = G                    # graphs
        self.NC = NC                  # cores
        self.D = 128
        self.NPC = N // NC            # real nodes per core
        self.BLK = 128
        self.NBLK = -(-self.NPC // self.BLK)       # blocks per core
        self.NPAD = self.NBLK * self.BLK           # padded nodes per core
        self.NPADTOT = self.NPAD * NC
        self.ROW = 258                # [nh 128 | h 128 | 1 | pad]
        self.SBS = blocks_per_sb      # dst blocks per super-block
        self.NSB = -(-self.NBLK // self.SBS)
        self.L = 3
        self.W = 16                   # per-core graph-id window (sorted ids
                                      # mean each core's nodes span few graphs)


# ---------------------------------------------------------------- host prep

def _prep(cfg, h, src, dst, graph_ids, betas):
    """Build per-core input maps + the shared tile schedule."""
    N, NC, NPC, NPAD, BLK, NBLK = cfg.N, cfg.NC, cfg.NPC, cfg.NPAD, cfg.BLK, cfg.NBLK
    h = np.asarray(h, np.float32)
    src = np.asarray(src, np.int64)
    dst = np.asarray(dst, np.int64)
    graph_ids = np.asarray(graph_ids, np.int64)
    betas = np.asarray(betas, np.float32)

    # edges sorted by global dst -> grouped by (core, local block)
    order = np.argsort(dst, kind="stable")
    e_src = src[order]
    e_dst = dst[order]
    src_pad = (e_src // NPC) * NPAD + (e_src % NPC)
    dcore = e_dst // NPC
    dlocal = e_dst % NPC
    dblk = dlocal // BLK

    # per (core, block) edge counts -> shared tile schedule
    cnt = np.zeros((NC, NBLK), np.int64)
    np.add.at(cnt, (dcore, dblk), 1)
    T_b = np.maximum(1, -(-cnt.max(0) // 128))     # tiles per block (shared)
    Ttot = int(T_b.sum())
    tcol0 = np.zeros(NBLK, np.int64)               # first tile col per block
    tcol0[1:] = np.cumsum(T_b)[:-1]

    sqbeta = np.zeros((128, 4), np.float32)
    for l in range(min(3, len(betas))):
        sqbeta[:, l] = np.sqrt(betas[l])

    # within-(core,block) rank of each edge (edges are sorted by dst so
    # (core, block) groups are contiguous and in order)
    blk_first = np.zeros((NC, NBLK), np.int64)     # first edge idx per group
    flat_cnt = cnt.reshape(-1)
    blk_first.reshape(-1)[1:] = np.cumsum(flat_cnt)[:-1]
    rank = np.arange(len(e_dst)) - blk_first[dcore, dblk]
    ecol = tcol0[dblk] + rank // 128               # tile column of each edge
    erow = rank % 128                              # tile row of each edge

    # one contiguous u8 blob per core: [h8 | srcidx u16 | drel i8 | gid | sqbeta]
    nbytes = NPC * 128 + 384 * Ttot + 128 * NBLK + 2048
    blob = np.zeros((NC, nbytes), np.uint8)
    w0s = []
    for c in range(NC):
        dummy = c * NPAD + NPC                     # an all-zero table row
        srcidx = np.full((128, Ttot), dummy, np.uint16)
        drel = np.full((128, Ttot), -1, np.int8)   # pad edges: mask misses

        m = dcore == c
        srcidx[erow[m], ecol[m]] = src_pad[m]
        drel[erow[m], ecol[m]] = dlocal[m] - dblk[m] * BLK

        h8 = h[c * NPC:(c + 1) * NPC].astype(ml_dtypes.float8_e3m4)

        gl = graph_ids[c * NPC:(c + 1) * NPC]
        w0 = int(gl[0])
        if int(gl[-1]) - w0 >= cfg.W:
            raise ValueError(
                f"core {c} graph span {int(gl[-1]) - w0 + 1} exceeds window "
                f"{cfg.W}; widen Cfg.W")
        w0s.append(w0)
        gid = np.full(NPAD, -1, np.int8)
        gid[:NPC] = gl - w0
        gid = np.ascontiguousarray(gid.reshape(NBLK, 128).T)  # [128, NBLK]

        parts = [h8.view(np.uint8).reshape(-1), srcidx.view(np.uint8).reshape(-1),
                 drel.view(np.uint8).reshape(-1), gid.view(np.uint8).reshape(-1),
                 sqbeta.view(np.uint8).reshape(-1)]
        blob[c] = np.concatenate(parts)

    counts = np.bincount(graph_ids, minlength=cfg.G).astype(np.float32)
    sched = dict(T_b=[int(x) for x in T_b], tcol0=[int(x) for x in tcol0],
                 Ttot=Ttot, nbytes=nbytes, w0s=w0s)
    return blob.reshape(NC * nbytes), counts, sched


# ---------------------------------------------------------------- device program

def build_program(cfg, sched, trace_sim=False):
    f16, f32, i32 = mybir.dt.float16, mybir.dt.float32, mybir.dt.int32
    u16, i8, f8 = mybir.dt.uint16, mybir.dt.int8, mybir.dt.float8e3
    T_b, tcol0 = sched["T_b"], sched["tcol0"]
    Ttot = sched["Ttot"]
    NBLK, SBS, NSB, ROW, G = cfg.NBLK, cfg.SBS, cfg.NSB, cfg.ROW, cfg.G
    Tmax = max(sum(T_b[sb * SBS:(sb + 1) * SBS]) for sb in range(NSB))

    nc = bacc.Bacc("TRN2", target_bir_lowering=False, debug=False,
                   num_devices=cfg.NC)

    nbytes = sched["nbytes"]
    W, NPC = cfg.W, cfg.NPC
    blob_d = nc.dram_tensor("blob", [nbytes], mybir.dt.uint8,
                            kind="ExternalInput").ap()
    o0 = NPC * 128
    o1 = o0 + 256 * Ttot
    o2 = o1 + 128 * Ttot
    o3 = o2 + 128 * NBLK
    h8_d = blob_d[0:o0].bitcast(f8).rearrange("(n d) -> n d", d=128)
    srcidx_d = blob_d[o0:o1].bitcast(u16).rearrange("(p t) -> p t", t=Ttot)
    drel_d = blob_d[o1:o2].bitcast(i8).rearrange("(p t) -> p t", t=Ttot)
    gid_d = blob_d[o2:o3].bitcast(i8).rearrange("(p b) -> p b", b=NBLK)
    sqbeta_d = blob_d[o3:o3 + 2048].bitcast(f32).rearrange("(p x) -> p x", x=4)
    pooled_d = nc.dram_tensor("pooled", [W, 128], f16, kind="ExternalOutput").ap()

    own = [nc.dram_tensor(f"own{l}", [cfg.NPAD, ROW], f16).ap()
           for l in range(cfg.L)]
    tab_space = "Shared" if cfg.NC > 4 else "Local"
    tabs = [nc.dram_tensor(f"tab{l}", [cfg.NPADTOT, ROW], f16,
                           addr_space=tab_space).ap()
            for l in range(cfg.L)]

    groups = [list(range(cfg.NC))]

    from contextlib import ExitStack

    with tile.TileContext(nc, trace_sim=trace_sim) as tc, ExitStack() as ctx:
        const = ctx.enter_context(tc.tile_pool(name="const", bufs=1))
        iota_i = const.tile([128, 128], i32)
        nc.gpsimd.iota(iota_i[:], pattern=[[1, 128]], base=0, channel_multiplier=0)
        iota_f = const.tile([128, 128], f16)
        nc.vector.tensor_copy(iota_f[:], iota_i[:])
        iotac_i = const.tile([128, 1], i32)
        nc.gpsimd.iota(iotac_i[:], pattern=[[0, 1]], base=0, channel_multiplier=1)
        iotac_f = const.tile([128, 1], f16)
        nc.vector.tensor_copy(iotac_f[:], iotac_i[:])
        # identity matrix for PE transposes: I[p, j] = (j == p)
        ident = const.tile([128, 128], f16)
        ic_b = iotac_f[:].rearrange("p (a b) -> p a b", a=1) \
            .to_broadcast([128, 1, 128])
        io_r = iota_f[:].rearrange("p (a b) -> p a b", a=1)
        nc.vector.tensor_tensor(
            out=ident[:].rearrange("p (a b) -> p a b", a=1),
            in0=io_r, in1=ic_b, op=mybir.AluOpType.is_equal)
        sqbeta = const.tile([128, 4], f32)
        nc.sync.dma_start(sqbeta[:], sqbeta_d)

        # selg[p, b*W + j] = (local_graph_id[b*128 + p] == j)
        gid8 = const.tile([128, NBLK], i8)
        nc.sync.dma_start(gid8[:], gid_d)
        selg_s = const.tile([128, NBLK * W], f16)
        sg3 = selg_s[:].rearrange("p (b j) -> p b j", j=W)
        io_g = iota_f[:, 0:W].rearrange("p (o j) -> p o j", o=1) \
            .to_broadcast([128, NBLK, W])
        gid_b = gid8[:].rearrange("p (b o) -> p b o", o=1) \
            .to_broadcast([128, NBLK, W])
        nc.vector.tensor_tensor(out=sg3, in0=io_g, in1=gid_b,
                                op=mybir.AluOpType.is_equal)

        idxp = ctx.enter_context(tc.tile_pool(name="idxp", bufs=3))
        gp = ctx.enter_context(tc.tile_pool(name="gp", bufs=2))
        cp = ctx.enter_context(tc.tile_pool(name="cp", bufs=2))
        ep = ctx.enter_context(tc.tile_pool(name="ep", bufs=2))
        pp = ctx.enter_context(tc.tile_pool(name="pp", bufs=2, space="PSUM"))
        pt = ctx.enter_context(tc.tile_pool(name="pt", bufs=2, space="PSUM"))
        pq = ctx.enter_context(tc.tile_pool(name="pq", bufs=2, space="PSUM"))
        ppool = ctx.enter_context(tc.tile_pool(name="ppool", bufs=1, space="PSUM"))

        pool_ps = ppool.tile([W, 128], f32, tag="pool")

        def build_rows(h3, out_dram, sb, nb, lidx):
            """Pack [nh*sqrt(beta_lidx) | h | 1] rows of super-block sb and
            DMA them to out_dram. h3: [128, nb, 128] f32 or f16 tile view."""
            sq = ep.tile([128, SBS * 128], f32, tag="sq")
            q3 = sq[:, 0:nb * 128].rearrange("p (b d) -> p b d", d=128)
            nc.vector.tensor_tensor(out=q3, in0=h3, in1=h3,
                                    op=mybir.AluOpType.mult)
            ss = ep.tile([128, SBS], f32, tag="ss")
            nc.vector.tensor_reduce(
                out=ss[:, 0:nb], in_=q3, axis=mybir.AxisListType.X,
                op=mybir.AluOpType.add)
            nrm = ep.tile([128, SBS], f32, tag="nrm")
            nc.scalar.sqrt(nrm[:, 0:nb], ss[:, 0:nb])
            nc.vector.tensor_scalar_add(nrm[:, 0:nb], nrm[:, 0:nb], EPS)
            rn = ep.tile([128, SBS], f32, tag="rn")
            nc.vector.reciprocal(rn[:, 0:nb], nrm[:, 0:nb])

            stg = ep.tile([128, SBS * ROW], f16, tag="stg")
            st3 = stg[:, 0:nb * ROW].rearrange("p (b d) -> p b d", d=ROW)
            rn_b = rn[:, 0:nb].rearrange("p (b o) -> p b o", o=1) \
                .to_broadcast([128, nb, 128])
            nc.vector.scalar_tensor_tensor(
                out=st3[:, :, 0:128], in0=h3,
                scalar=sqbeta[:, lidx:lidx + 1], in1=rn_b,
                op0=mybir.AluOpType.mult, op1=mybir.AluOpType.mult)
            nc.scalar.copy(out=st3[:, :, 128:256], in_=h3)
            nc.vector.memset(st3[:, :, 256:258], 1.0)
            out_ap = out_dram[sb * SBS * 128: sb * SBS * 128 + nb * 128, :] \
                .rearrange("(b p) d -> p b d", p=128)
            nc.sync.dma_start(out_ap, st3)

        # ---- layer-0 table build: widen fp8, normalize + pack, AllGather
        # (h8 ships only the NPC real rows; the tail of the last super-block
        # is zero-filled so pad rows become all-zero table rows)
        for sb in range(NSB):
            blocks = list(range(sb * SBS, min((sb + 1) * SBS, NBLK)))
            nb = len(blocks)
            r0 = sb * SBS * 128
            nreal = min(nb * 128, NPC - r0)
            nfull = nreal // 128
            rem = nreal % 128
            hin8 = ep.tile([128, SBS * 128], f8, tag="hin8")
            h83 = hin8[:, 0:nb * 128].rearrange("p (b d) -> p b d", d=128)
            hin = ep.tile([128, SBS * 128], f16, tag="hin")
            hi3 = hin[:, 0:nb * 128].rearrange("p (b d) -> p b d", d=128)
            if nreal < nb * 128:
                nc.vector.memset(hi3, 0.0)
            if nfull:
                nc.sync.dma_start(
                    h83[:, 0:nfull, :],
                    h8_d[r0: r0 + nfull * 128, :]
                    .rearrange("(b p) d -> p b d", p=128))
                # fp8e3 -> f16 must run on the scalar engine (DVE e3m4 path
                # is broken on HW)
                nc.scalar.copy(out=hi3[:, 0:nfull, :], in_=h83[:, 0:nfull, :])
            if rem:
                nc.sync.dma_start(
                    h83[0:rem, nfull, :],
                    h8_d[r0 + nfull * 128: r0 + nreal, :])
                nc.scalar.copy(out=hi3[0:rem, nfull, :],
                               in_=h83[0:rem, nfull, :])
            build_rows(hi3, own[0], sb, nb, 0)
        nc.gpsimd.collective_compute(
            "AllGather", mybir.AluOpType.bypass, replica_groups=groups,
            ins=[own[0][:, :]], outs=[tabs[0][:, :]])

        for l in range(cfg.L):
            tab = tabs[l]
            for sb in range(NSB):
                blocks = list(range(sb * SBS, min((sb + 1) * SBS, NBLK)))
                nb = len(blocks)
                c0 = tcol0[blocks[0]]
                Tsb = sum(T_b[b] for b in blocks)

                tile_bi = []
                for bi, b in enumerate(blocks):
                    tile_bi += [bi] * T_b[b]

                # ---- indices (u16 -> i32 on device) + dst-rel positions
                idxu = idxp.tile([128, Tmax], u16, tag="idxu")
                nc.sync.dma_start(idxu[:, 0:Tsb], srcidx_d[:, c0:c0 + Tsb])
                idx_s = idxp.tile([128, Tmax], i32, tag="idxs")
                nc.vector.tensor_copy(idx_s[:, 0:Tsb], idxu[:, 0:Tsb])
                drel = idxp.tile([128, Tmax], i8, tag="drel")
                nc.sync.dma_start(drel[:, 0:Tsb], drel_d[:, c0:c0 + Tsb])

                # ---- local dst nh rows for this super-block
                nhblk = idxp.tile([128, SBS * 128], f16, tag="nhblk")
                nb3 = nhblk[:, 0:nb * 128].rearrange("p (b d) -> p b d", d=128)
                nc.sync.dma_start(
                    nb3,
                    own[l][sb * SBS * 128: sb * SBS * 128 + nb * 128, 0:128]
                    .rearrange("(b p) d -> p b d", p=128))

                # ---- src gather: one [128,1]-offset call per 128-edge tile
                # (HW contract: partition p reads a contiguous line from
                # row idx[p]; multi-column offset APs are NOT honored)
                gsrc = gp.tile([128, Tmax * ROW], f16, tag="gsrc")
                for t in range(Tsb):
                    nc.gpsimd.indirect_dma_start(
                        out=gsrc[:, t * ROW:(t + 1) * ROW], out_offset=None,
                        in_=tab, in_offset=bass.IndirectOffsetOnAxis(
                            ap=idx_s[:, t:t + 1], axis=0))
                g3 = gsrc[:, 0:Tsb * ROW].rearrange("p (t d) -> p t d", d=ROW)

                # ---- edge mask:  sel[e, t, j] = (iota[j] == drel[e, t])
                sel = gp.tile([128, Tmax * 128], f16, tag="sel")
                s3 = sel[:, 0:Tsb * 128].rearrange("p (t j) -> p t j", j=128)
                io_b = iota_f[:].rearrange("p (o j) -> p o j", o=1) \
                    .to_broadcast([128, Tsb, 128])
                dr_b = drel[:, 0:Tsb].rearrange("p (t o) -> p t o", o=1) \
                    .to_broadcast([128, Tsb, 128])
                nc.vector.tensor_tensor(
                    out=s3, in0=io_b, in1=dr_b, op=mybir.AluOpType.is_equal)

                # ---- scores: transpose the mask on the PE, pull each
                # edge's dst nh row, then s = <nh_src, nh_dst>, a = exp(s)
                s_t = cp.tile([128, Tmax], f32, tag="s")
                for g0 in range(0, Tsb, 4):
                    gn = min(4, Tsb - g0)
                    mT = pt.tile([128, 512], f32, tag="mT")
                    for k in range(gn):
                        nc.tensor.matmul(
                            out=mT[:, k * 128:(k + 1) * 128],
                            lhsT=s3[:, g0 + k, :], rhs=ident[:],
                            start=True, stop=True)
                    sT = cp.tile([128, 512], f16, tag="sT")
                    nc.scalar.copy(out=sT[:, 0:gn * 128], in_=mT[:, 0:gn * 128])
                    px = pq.tile([128, 512], f32, tag="px")
                    for k in range(gn):
                        nc.tensor.matmul(
                            out=px[:, k * 128:(k + 1) * 128],
                            lhsT=sT[:, k * 128:(k + 1) * 128],
                            rhs=nb3[:, tile_bi[g0 + k], :],
                            start=True, stop=True)
                    prod = cp.tile([128, 512], f16, tag="prod")
                    pr3 = prod[:, 0:gn * 128].rearrange("p (t d) -> p t d", d=128)
                    nc.vector.tensor_tensor(
                        out=pr3,
                        in0=px[:, 0:gn * 128].rearrange("p (t d) -> p t d", d=128),
                        in1=g3[:, g0:g0 + gn, 0:128], op=mybir.AluOpType.mult)
                    nc.vector.tensor_reduce(
                        out=s_t[:, g0:g0 + gn], in_=pr3,
                        axis=mybir.AxisListType.X, op=mybir.AluOpType.add)
                a_t = cp.tile([128, Tmax], f16, tag="a")
                nc.scalar.activation(
                    out=a_t[:, 0:Tsb], in_=s_t[:, 0:Tsb],
                    func=mybir.ActivationFunctionType.Exp)

                # ---- attention weights:  sel *= a  (in place)
                a_b = a_t[:, 0:Tsb].rearrange("p (t o) -> p t o", o=1) \
                    .to_broadcast([128, Tsb, 128])
                nc.vector.tensor_tensor(
                    out=s3, in0=s3, in1=a_b, op=mybir.AluOpType.mult)

                # ---- scatter:  psum[:, bb*129:(bb+1)*129] += asel_t^T @ [h|1]
                pn = pp.tile([128, 512], f32, tag="pn")
                tt = 0
                for bi, b in enumerate(blocks):
                    for t in range(T_b[b]):
                        nc.tensor.matmul(
                            out=pn[:, bi * 129:bi * 129 + 129],
                            lhsT=s3[:, tt, :],
                            rhs=g3[:, tt, 128:257],
                            start=(t == 0), stop=(t == T_b[b] - 1))
                        tt += 1

                # ---- epilogue: h' = num / max(den, tiny)
                p3 = pn[:, 0:nb * 129].rearrange("p (b d) -> p b d", d=129)
                den = ep.tile([128, SBS], f32, tag="den")
                nc.vector.tensor_scalar_max(den[:, 0:nb], p3[:, :, 128:129], 1e-30)
                rec = ep.tile([128, SBS], f32, tag="rec")
                nc.vector.reciprocal(rec[:, 0:nb], den[:, 0:nb])
                hsb = ep.tile([128, SBS * 128], f32, tag="hsb")
                h3 = hsb[:, 0:nb * 128].rearrange("p (b d) -> p b d", d=128)
                rec_b = rec[:, 0:nb].rearrange("p (b o) -> p b o", o=1) \
                    .to_broadcast([128, nb, 128])
                nc.vector.tensor_tensor(
                    out=h3, in0=p3[:, :, 0:128], in1=rec_b,
                    op=mybir.AluOpType.mult)

                if l < cfg.L - 1:
                    build_rows(h3, own[l + 1], sb, nb, l + 1)
                else:
                    hf = ep.tile([128, SBS * 128], f16, tag="hf")
                    hf3 = hf[:, 0:nb * 128].rearrange("p (b d) -> p b d", d=128)
                    nc.scalar.copy(out=hf3, in_=h3)
                    for bi, b in enumerate(blocks):
                        nc.tensor.matmul(
                            out=pool_ps[:, :],
                            lhsT=selg_s[:, b * W:b * W + W],
                            rhs=hf3[:, bi, :],
                            start=(b == 0), stop=(b == NBLK - 1))

            if l < cfg.L - 1:
                nc.gpsimd.collective_compute(
                    "AllGather", mybir.AluOpType.bypass,
                    replica_groups=groups,
                    ins=[own[l + 1][:, :]], outs=[tabs[l + 1][:, :]])

        pooled_s = const.tile([W, 128], f16)
        nc.scalar.copy(out=pooled_s[:, :], in_=pool_ps[:, :])
        nc.sync.dma_start(pooled_d, pooled_s[:, :])

    return nc


# ---------------------------------------------------------------- runner

LAST_EXEC_NS = None
_CACHE = {}


def _build_runner(nc, n_cores):
    """A cached PJRT executable for nc: jit once, reuse across runs.
    Each call still uploads the full per-core inputs and downloads the
    outputs (matching run_bass_kernel_spmd's per-call semantics)."""
    import jax
    from jax.sharding import Mesh, PartitionSpec
    from jax.experimental.shard_map import shard_map
    from concourse.bass2jax import (
        _bass_exec_p, partition_id_tensor, install_neuronx_cc_hook)

    install_neuronx_cc_hook()
    assert nc.dbg_addr is None or not nc.dbg_callbacks

    partition_name = nc.partition_id_tensor.name if nc.partition_id_tensor else None
    in_names, out_names, out_avals = [], [], []
    for alloc in nc.m.functions[0].allocations:
        if not isinstance(alloc, mybir.MemoryLocationSet):
            continue
        name = alloc.memorylocations[0].name
        if alloc.kind == "ExternalInput":
            if name != partition_name:
                in_names.append(name)
        elif alloc.kind == "ExternalOutput":
            out_names.append(name)
            out_avals.append(jax.core.ShapedArray(
                tuple(alloc.tensor_shape), mybir.dt.np(alloc.dtype)))
    n_params = len(in_names)
    n_outs = len(out_avals)
    all_in_names = in_names + out_names + (
        [partition_name] if partition_name else [])
    donate = tuple(range(n_params, n_params + n_outs))

    def _body(*args):
        operands = list(args)
        if partition_name is not None:
            operands.append(partition_id_tensor())
        outs = _bass_exec_p.bind(
            *operands, out_avals=tuple(out_avals), in_names=tuple(all_in_names),
            out_names=tuple(out_names), lowering_input_output_aliases=(),
            sim_require_finite=True, sim_require_nnan=True, nc=nc)
        return tuple(outs)

    devices = jax.devices()[:n_cores]
    mesh = Mesh(np.asarray(devices), ("core",))
    sharded = jax.jit(
        shard_map(_body, mesh=mesh,
                  in_specs=(PartitionSpec("core"),) * (n_params + n_outs),
                  out_specs=(PartitionSpec("core"),) * n_outs,
                  check_rep=False),
        donate_argnums=donate, keep_unused=True)

    # The custom-call convention takes (donated) output buffers as trailing
    # operands. The program fully overwrites them, so their contents are
    # irrelevant: recycle the previous call's device-resident outputs as the
    # next call's donation targets, so only the real inputs cross the wire.
    prev_out = [None]

    def run(full_map):
        """full_map: input name -> pre-concatenated (n_cores*dim0, ...) array."""
        concat_in = [np.asarray(full_map[name]) for name in in_names]
        if prev_out[0] is None:
            prev_out[0] = [
                np.zeros((n_cores * a.shape[0], *a.shape[1:]), a.dtype)
                for a in out_avals]
        out_arrs = sharded(*concat_in, *prev_out[0])
        res = [
            {name: np.asarray(out_arrs[i]).reshape(
                n_cores, *out_avals[i].shape)[c]
             for i, name in enumerate(out_names)}
            for c in range(n_cores)]
        prev_out[0] = list(out_arrs)
        return res

    return run


def _get_runner(cfg, sched):
    key = tuple(sched["T_b"])
    if key not in _CACHE:
        nc = build_program(cfg, sched)
        nc.compile()
        _CACHE[key] = (nc, _build_runner(nc, cfg.NC))
    return _CACHE[key][1]


# ---------------------------------------------------------------- entry

def kernel(h, src, dst, graph_ids, betas, W_cls, b_cls, time_execs=0):
    global LAST_EXEC_NS
    import time as _time

    cfg = Cfg(N=40000, E=640000, G=64, NC=8)
    blob, counts, sched = _prep(cfg, h, src, dst, graph_ids, betas)
    run = _get_runner(cfg, sched)
    full_map = {"blob": blob}

    def _run():
        last = None
        for attempt in range(3):
            try:
                return run(full_map)
            except Exception as e:  # transient axon worker hangs
                last = e
                _time.sleep(5)
        raise last

    res = _run()
    if time_execs:
        # no NTFF profiling hook is available in this container, so report
        # median wall-clock of repeated NEFF executions (includes the axon
        # dispatch + input-upload overhead; on-device time is lower)
        ts = []
        for _ in range(time_execs):
            t0 = _time.time()
            res = run(full_map)
            ts.append(_time.time() - t0)
        LAST_EXEC_NS = int(np.median(ts) * 1e9)
    pooled = np.zeros((cfg.G, 128), np.float64)
    for c, r in enumerate(res):
        lo = sched["w0s"][c]
        hi = min(lo + cfg.W, cfg.G)
        pooled[lo:hi] += r["pooled"][:hi - lo].astype(np.float64)
    hg = (pooled / np.maximum(counts, 1.0)[:, None]).astype(np.float32)
    return hg @ np.asarray(W_cls, np.float32) + np.asarray(b_cls, np.float32)
